# revision 1
# baseline (speedup 1.0000x reference)
"""CapsuleLayer dynamic-routing kernel for Trainium2 (8 NeuronCores).

Problem: x [256,1152,8] f32, route_weights [10,1152,8,16] f32 ->
out [10,256,1,16] f32 (3 routing iterations, softmax over the 1152
route nodes).

Algebra: logits accumulate additively and each delta is priors .
outputs_t, so logits_t = priors . u_t with u_1 = O_0, u_2 = O_0 + O_1.
Priors are never materialized; each iteration computes V = W_c @ u
(PE, fp16), evacuates V to SBUF fp16 (ACT), l = sum_i x * V (DVE fp16
2x mul + packed tree-add), e = exp(l) with fused per-partition sum d
(ACT), y^T = x^T * e^T (DVE/GPSIMD, bf16, broadcast over i),
s^T = sum_k W[k,o] y^T[k,b] (PE, 72 accumulated bf16 matmuls), then
squash out = S*sqrt(Q)/(d^2+Q) with Q = sum_o S^2 (DVE fused
mul-reduce) and sqrt(Q) = exp(0.5*ln Q) so every ACT function stays
in one activation-table set (no LoadActFuncSet churn).

Sharding: 20 units of (capsule c, batch-half of 128).  Each core gets a
batch-half and 3 c-slots (cores with only 2 real units get a dummy
all-ones weight whose output is discarded).  No cross-core comms.
x^T / W^T layouts and the 16-bit casts are prepared on the host.
"""

import sys

for _p in ("/opt/trn_rl_repo",):
    if _p not in sys.path:
        sys.path.insert(0, _p)

import numpy as np
from contextlib import ExitStack

import concourse.bass as bass
import concourse.tile as tile
from concourse import mybir
from concourse._compat import with_exitstack
from concourse.masks import make_identity

F32 = mybir.dt.float32
F16 = mybir.dt.float16
BF16 = mybir.dt.bfloat16
AF = mybir.ActivationFunctionType
OP = mybir.AluOpType
AX = mybir.AxisListType

C, B, R, I, O = 10, 256, 1152, 8, 16
K = R * I            # 9216
RB = R // 128        # 9 r-blocks
KT = K // 128        # 72 k-tiles
BH = 128             # batch-half per core
NSLOT = 3            # c-slots per core
NCORES = 8

# core k -> (batch_half, [c0, c1, c2]) ; -1 = dummy slot
CSETS = [[0, 4, 8], [1, 5, 9], [2, 6, -1], [3, 7, -1]]

import os
# chunk ids whose tree-adds run on GPSIMD instead of DVE
POOL_CHUNKS = set(
    int(t) for t in os.environ.get("CAPS_POOL_CHUNKS", "").split(",") if t)
# y-mul quarters assigned to GPSIMD
POOL_YQ = set(
    int(t) for t in os.environ.get("CAPS_POOL_YQ", "").split(",") if t)


def core_assignment(k):
    return k // 4, CSETS[k % 4]


@with_exitstack
def _caps_kernel(ctx: ExitStack, tc: tile.TileContext, out_ap, xh, xT_in,
                 w_aps, wT_aps):
    nc = tc.nc

    singles = ctx.enter_context(tc.tile_pool(name="singles", bufs=1))
    v_pool = ctx.enter_context(tc.tile_pool(name="vpool", bufs=4))
    tw_pool = ctx.enter_context(tc.tile_pool(name="twave", bufs=3))
    y_pool = ctx.enter_context(tc.tile_pool(name="yhalf", bufs=3))
    le_pool = ctx.enter_context(tc.tile_pool(name="le", bufs=2))
    e_pool = ctx.enter_context(tc.tile_pool(name="epool", bufs=2))
    et_pool = ctx.enter_context(tc.tile_pool(name="et", bufs=2))
    small = ctx.enter_context(tc.tile_pool(name="small", bufs=3))
    psv = ctx.enter_context(tc.tile_pool(name="psv", bufs=2, space="PSUM"))
    pst = ctx.enter_context(tc.tile_pool(name="pst", bufs=2, space="PSUM"))
    pss = ctx.enter_context(tc.tile_pool(name="pss", bufs=2, space="PSUM"))

    ident = singles.tile([128, 128], F32)
    make_identity(nc, ident)

    # ---- x^T (host-pretransposed, bf16): [p=r_off, i, rb, b], chunked by
    #      i, interleaved with the packed per-slot bf16 weights so
    #      iteration 0's matmul chain can start as soon as chunks land ----
    xT = singles.tile([128, I, RB, 128], F16)
    w_all = singles.tile([128, I, RB, 48], F16)
    for ip in range(4):
        nc.sync.dma_start(xT[:, 2 * ip:2 * ip + 2], xT_in[:, 2 * ip:2 * ip + 2])
        nc.sync.dma_start(w_all[:, 2 * ip:2 * ip + 2],
                          w_aps[:, 2 * ip:2 * ip + 2])

    # w_cT: [96, 9216] fp16; slot s at partitions 32s..32s+16, (r,i)-flat.
    # Slot 0 first so iteration 1 can start while the rest stream in.
    w_cT = singles.tile([96, K], F16)
    nc.sync.dma_start(w_cT[0:16, :], wT_aps[0])

    # ---- x natural layout [b, r, i] fp16 (contiguous per partition) ----
    x_u = singles.tile([128, R, I], F16)
    nc.sync.dma_start(x_u, xh)
    for s in range(1, NSLOT):
        nc.sync.dma_start(w_cT[32 * s:32 * s + 16, :], wT_aps[s])

    # u^T per slot lives at partitions 32s..32s+16 of one [96,128] fp16 tile
    uT = singles.tile([96, 128], F16)

    u_tiles = [None] * NSLOT

    def squash_tail(s, it, S_ap, d_ap):
        """S_ap [128,16] (psum, unnormalized s*d) + per-partition denom d
        -> Ot = S*sqrt(Q)/(d^2+Q); update u / uT / out."""
        sN = small.tile([128, O], F32, tag="sN")
        nc.vector.tensor_copy(sN, S_ap)
        scr = small.tile([128, O], F32, tag="scr")
        nc.vector.tensor_mul(scr, sN, sN)
        q = small.tile([128, 1], F32, tag="q")
        nc.vector.reduce_sum(q, scr, axis=AX.X)
        rq = small.tile([128, 1], F32, tag="rq")
        nc.scalar.sqrt(rq, q)
        den = small.tile([128, 1], F32, tag="den")
        if d_ap is None:
            nc.vector.tensor_scalar_add(den, q, float(R) * float(R))
        else:
            d2 = small.tile([128, 1], F32, tag="d2")
            nc.vector.tensor_mul(d2, d_ap, d_ap)
            nc.vector.tensor_add(den, d2, q)
        rden = small.tile([128, 1], F32, tag="rden")
        nc.vector.reciprocal(rden, den)
        gf = small.tile([128, 1], F32, tag="gf")
        nc.vector.tensor_mul(gf, rq, rden)
        Ot = small.tile([128, O], F32, tag=f"O{it}_{s}", bufs=1)
        nc.vector.tensor_scalar_mul(Ot, sN, gf)

        if it == 2:
            nc.sync.dma_start(out_ap[s], Ot)
            return
        if it == 0:
            u_tiles[s] = Ot
        else:
            u2 = small.tile([128, O], F32, tag=f"u2_{s}", bufs=1)
            nc.vector.tensor_add(u2, u_tiles[s], Ot)
            u_tiles[s] = u2
        psu = pst.tile([128, 512], F32, tag="ptr")
        nc.tensor.transpose(psu[0:16, 0:128], u_tiles[s], ident)
        ustg = small.tile([16, 128], F16, tag="ustg")
        nc.scalar.copy(ustg, psu[0:16, 0:128])
        nc.sync.dma_start(uT[32 * s:32 * s + 16, :], ustg)

    # ---- iteration 0: all 3 slots in one packed bf16 matmul chain ----
    ps48 = pss.tile([48, 128], F32, tag="pss")
    for idx in range(KT):
        i, rb = idx // RB, idx % RB
        nc.tensor.matmul(
            ps48, lhsT=w_all[:, i, rb, :], rhs=xT[:, i, rb, :],
            start=(idx == 0), stop=(idx == KT - 1),
        )
    sT48 = small.tile([48, 128], F32, tag="sT48")
    nc.scalar.copy(sT48, ps48)
    ps2a = pst.tile([128, 512], F32, tag="ptr")
    nc.tensor.transpose(ps2a[:, 0:48], sT48, ident[0:48, 0:48])
    for s in range(NSLOT):
        squash_tail(s, 0, ps2a[:, 16 * s:16 * s + 16], None)

    # ---- iterations 1, 2: software-pipelined across slots.
    # Stage A(s): V-matmul + ACT evacuate + fp16 x*V + tree-add (DVE).
    # Stage B(s): e^T transposes, y = x^T*e^T, s-matmul, squash.
    # B(s) steps are interleaved into A(s+1)'s chunk loop so the
    # per-engine static schedule overlaps the stages.
    def stage_B(it, s, e_t, d):
        eT = et_pool.tile([128, RB, 128], F16, tag="eT")
        eTf = eT.rearrange("p rb b -> p (rb b)")
        for g, cnt in ((0, 4), (4, 4), (8, 1)):
            ps = pst.tile([128, 512], F32, tag="ptr")
            for sub in range(cnt):
                rb = g + sub
                nc.tensor.transpose(
                    ps[:, sub * 128:(sub + 1) * 128],
                    e_t[:, rb * 128:(rb + 1) * 128], ident,
                )
            nc.scalar.copy(eTf[:, g * 128:(g + cnt) * 128],
                           ps[:, 0:cnt * 128])
            yield
        ps_s = pss.tile([16, 128], F32, tag="pss")
        for qq in range(4):
            yh = y_pool.tile([128, 2, RB, 128], F16, tag="yh")
            e_bcast = bass.AP(
                tensor=eT.tensor, offset=eT.offset,
                ap=[eT.ap[0], [0, 2], [128, RB], [1, 128]],
            )
            mul_eng = nc.gpsimd if qq in POOL_YQ else nc.vector
            mul_eng.tensor_mul(yh, xT[:, qq * 2:(qq + 1) * 2, :, :], e_bcast)
            for jj in range(18):
                ii, rb = jj // RB, jj % RB
                idx = qq * 18 + jj
                nc.tensor.matmul(
                    ps_s,
                    lhsT=w_all[:, qq * 2 + ii, rb, 16 * s:16 * s + 16],
                    rhs=yh[:, ii, rb, :],
                    start=(idx == 0), stop=(idx == KT - 1),
                )
            yield
        sT_sb = small.tile([16, 128], F32, tag="sTsb")
        nc.scalar.copy(sT_sb, ps_s)
        ps2 = pst.tile([128, 512], F32, tag="ptr")
        nc.tensor.transpose(ps2[:, 0:16], sT_sb, ident[0:16, 0:16])
        squash_tail(s, it, ps2[:, 0:16], d)
        yield

    def drain(gen, n=None):
        if gen is None:
            return None
        try:
            if n is None:
                while True:
                    next(gen)
            else:
                for _ in range(n):
                    next(gen)
        except StopIteration:
            return None
        return gen

    pending = None
    for it in (1, 2):
        for s in range(NSLOT):
            l_t = le_pool.tile([128, R], F16, tag="l")
            for w9 in range(9):
                pv = psv.tile([128, 1024], F32, tag="pv")
                for cc in range(2):
                    ck = w9 * 2 + cc
                    nc.tensor.matmul(
                        pv[:, cc * 512:(cc + 1) * 512],
                        lhsT=uT[32 * s:32 * s + 16, :],
                        rhs=w_cT[32 * s:32 * s + 16,
                                 ck * 512:(ck + 1) * 512],
                        start=True, stop=True,
                    )
                Vs = v_pool.tile([128, 1024], F16, tag="vs")
                nc.scalar.copy(Vs, pv)
                tw = tw_pool.tile([128, 128, I], F16, tag="tw")
                nc.vector.tensor_mul(
                    tw, x_u[:, w9 * 128:(w9 + 1) * 128, :],
                    Vs.rearrange("p (r i) -> p r i", i=I),
                )
                eng = nc.gpsimd if w9 in POOL_CHUNKS else nc.vector
                t4 = tw_pool.tile([128, 128, 4], F16, tag="t4")
                eng.tensor_add(t4, tw[:, :, 0:4], tw[:, :, 4:8])
                t2 = tw_pool.tile([128, 128, 2], F16, tag="t2")
                eng.tensor_add(t2, t4[:, :, 0:2], t4[:, :, 2:4])
                eng.tensor_add(l_t[:, w9 * 128:(w9 + 1) * 128],
                               t2[:, :, 0:1], t2[:, :, 1:2])
                pending = drain(pending, 1)
            pending = drain(pending)
            # stable softmax: e = exp(l - max l); squash is scale-invariant
            # in (S, d) so the shift cancels exactly.  Keeps e in bf16 range
            # and keeps Q = |S|^2 inside the ACT ln table's domain (2^64).
            negm = small.tile([128, 1], F32, tag="negm")
            nc.vector.reduce_max(negm, l_t, axis=AX.X, negate=True)
            d = small.tile([128, 1], F32, tag="d")
            e_t = e_pool.tile([128, R], F32, tag="e")
            nc.scalar.activation(e_t, l_t, AF.Exp, bias=negm, accum_out=d)
            pending = stage_B(it, s, e_t, d)
    drain(pending)


def build_program():
    from concourse import bacc

    # Steer the activation-table chooser to the one set that holds every
    # function this kernel uses (exp, ln, copy, identity, square), so the
    # program loads a single table instead of ping-ponging between the
    # exp-only and ln-only sets (1.28us per reload on the ACT engine).
    # Only membership is masked; set ids keep their act_info.json indices,
    # so the emitted LoadActFuncSet ids stay valid for the HW compile.
    if os.environ.get("CAPS_ACT_PATCH", "") == "1" and \
            not getattr(bacc, "_caps_act_tables_patched", False):
        _orig_get_tables = bacc.get_activation_tables

        def _one_set_tables(arch):
            tabs = dict(_orig_get_tables(arch))
            keep = "natural_log_exp_and_others"
            if keep in tabs:
                tabs = {name: (funcs if name == keep else set())
                        for name, funcs in tabs.items()}
            return tabs

        bacc.get_activation_tables = _one_set_tables
        bacc._caps_act_tables_patched = True

    nc = bacc.Bacc("TRN2", target_bir_lowering=False, debug=False,
                   num_devices=NCORES)
    xh = nc.declare_dram_parameter("xh", [BH, R, I], F16, isOutput=False).ap()
    xT_in = nc.declare_dram_parameter("xT", [128, I, RB, BH], F16,
                                      isOutput=False).ap()
    w_aps = nc.declare_dram_parameter("wall", [128, I, RB, 48], F16,
                                      isOutput=False).ap()
    wT_aps = [
        nc.declare_dram_parameter(f"wT{s}", [O, K], F16, isOutput=False).ap()
        for s in range(NSLOT)
    ]
    out = nc.declare_dram_parameter("out", [NSLOT, BH, O], F32,
                                    isOutput=True).ap()
    with tile.TileContext(nc) as tc:
        _caps_kernel(tc, out, xh, xT_in, w_aps, wT_aps)
    nc.compile()
    return nc


def make_in_maps(x, w):
    in_maps = []
    ones_w = np.ones([R, I, O], dtype=np.float32)
    xTs = {}
    xhs = {}
    for h in range(2):
        xh_np = x[h * BH:(h + 1) * BH]  # [128 b, 1152 r, 8 i]
        # xT[p=r_off, i, rb, b] = xh[b, rb*128+p, i]
        xTs[h] = np.ascontiguousarray(
            xh_np.reshape(BH, RB, 128, I).transpose(2, 3, 1, 0)).astype(
                np.float16)
        xhs[h] = np.ascontiguousarray(xh_np).astype(np.float16)
    wTs = {c: np.ascontiguousarray(w[c].reshape(K, O).T).astype(np.float16)
           for c in range(C)}
    wT_ones = np.ascontiguousarray(ones_w.reshape(K, O).T).astype(np.float16)
    # packed [p, i, rb, 16s+o] = w[c_s][rb*128+p, i, o]
    w_pk = {c: w[c].reshape(RB, 128, I, O).transpose(1, 2, 0, 3)
            for c in range(C)}
    ones_pk = ones_w.reshape(RB, 128, I, O).transpose(1, 2, 0, 3)
    for k in range(NCORES):
        h, cs = core_assignment(k)
        m = {"xh": xhs[h], "xT": xTs[h]}
        wall = np.concatenate(
            [w_pk[c] if c >= 0 else ones_pk for c in cs], axis=3)
        m["wall"] = np.ascontiguousarray(wall).astype(np.float16)
        for s, c in enumerate(cs):
            m[f"wT{s}"] = wTs[c] if c >= 0 else wT_ones
        in_maps.append(m)
    return in_maps


def kernel(x: np.ndarray, route_weights: np.ndarray) -> np.ndarray:
    from concourse.bass_utils import run_bass_kernel_spmd

    x = np.ascontiguousarray(x, dtype=np.float32)
    w = np.ascontiguousarray(route_weights, dtype=np.float32)
    in_maps = make_in_maps(x, w)
    nc = build_program()
    res = run_bass_kernel_spmd(nc, in_maps, list(range(NCORES)))
    global LAST_RESULTS
    LAST_RESULTS = res

    out = np.zeros([C, B, 1, O], dtype=np.float32)
    for k in range(NCORES):
        h, cs = core_assignment(k)
        o = res.results[k]["out"]
        for s, c in enumerate(cs):
            if c >= 0:
                out[c, h * BH:(h + 1) * BH, 0, :] = o[s]
    return out


if __name__ == "__main__":
    rng = np.random.default_rng(0)
    x = rng.normal(size=(B, R, I)).astype(np.float32)
    w = rng.normal(size=(C, R, I, O)).astype(np.float32)
    out = kernel(x=x, route_weights=w)
    print(out.shape, out.dtype, np.abs(out).mean())

